# revision 44
# baseline (speedup 1.0000x reference)
"""BD3LM block-diffusion decoder layer on 8 trn2 NeuronCores.

Sharding: core = 2*b + g  (b = batch 0..3, g = head-group 0..1, 8 heads each).
Each core: QKV projections for its batch/head-group, sparse BD3LM attention
(only ~80 of 256 score tiles per head), O-projection against its Wo row-slice.
Host: sums the two group partials per batch and adds the (bv @ Wo + bo)
correction (softmax rows sum to 1, so the v-bias contributes exactly bv @ Wo).

All matmuls fp16, fp32 PSUM accumulation. DRAM tensors are pre-swizzled on
the host so every DMA reads contiguous per-partition lines (no strided
gathers): xT [P, KC, T], weights [P, KC, DG] / [P, DT4, D].

Attention processes head pairs (2c, 2c+1) with the even head's K=64 score
matmuls on array rows 0-63 and the odd head's on 64-127, emitted ADJACENT
so the two row groups execute concurrently (HW-measured 101 ns vs 561 ns
per [*,512] matmul when a row group repeats back-to-back). Per (c, q-bank)
block: score runs (64-row tiling mode) fill double-bank [128, 1024] PSUM
pairs (even head cols 0-511, odd 512-1023), each pair exp'd by a SINGLE
activation and mask-multiplied by a single DVE op; ctx runs (128-row mode)
accumulate per-half [128, 1024] pairs (row 64 = softmax denominator from a
ones column in v). Grouping same-tiling-mode matmuls minimizes PE array
drains on mode switches.

Scheduling: blocks are software-pipelined — block B+1's ACT-bound score
stages emit between block B's PE-dense ctx stages; the normalize broadcast
(PE, gated on a DVE reciprocal) defers past two score j's so PE never
waits on the recip; qk(1..3) and completed O-proj tiles pop as fillers
inside score runs; and each REPEAT iteration's prologue (x DMAs, V pass,
qk(0)) pops inside the previous iteration's tail (x and v_x0 ping-pong),
so the steady-state stream keeps the PE continuously fed — this also keeps
the HAM clock-gate warm. Normalization: one [1,1024] reciprocal + one PE
broadcast pair + one copy + two muls per (c, half, qb); odd-head results
reach ctxT partitions 64-127 via a small HWDGE SBUF-to-SBUF DMA.
"""

import numpy as np

import concourse.bass as bass
import concourse.mybir as mybir
import concourse.tile as tile
from concourse import bacc
from concourse.bass_utils import run_bass_kernel_spmd

F32 = mybir.dt.float32
F16 = mybir.dt.float16
Act = mybir.ActivationFunctionType

B, T, D = 4, 2048, 1024
H, HD = 16, 64
L = T // 2           # 1024, length of each of [xt | x0]
BS = 4               # block size
G = 2                # head groups (cores per batch)
DG = D // G          # 512 channels per group
HG = H // G          # 8 heads per core
P = 128
NT = L // P          # 8 key/query tiles per half
KC = D // P          # 8 contraction chunks
DT4 = DG // P        # 4 head-pair chunks (128 ch each)

REPEAT = 1  # loop whole computation inside the NEFF (timing experiments only)
PHASES = "all"  # "all" | comma list of v,qk,attn,oproj + experiment flags
TIMING = False  # True: identical compute, but big I/O tensors become
# Internal DRAM (content-independent timing) + a tiny external output, so
# sync wall-clock per call is the ~84ms axon RPC floor (sigma ~0.2ms)
# instead of ~92ms +- 4ms of input shipping. Used only for measurement.

_CACHE = {}


def _on(name):
    return PHASES == "all" or name in PHASES.split(",")


def _x(name):
    """Experiment-only modifier: never active in the real kernel."""
    return PHASES != "all" and name in PHASES.split(",")


def _build():
    import concourse.tile_utils as tile_utils

    tile_utils.max_sbuf_usage = 207 * 1024

    nc = bacc.Bacc("TRN2", target_bir_lowering=False, debug=False, num_devices=8)

    kin = "Internal" if TIMING else "ExternalInput"
    kout = "Internal" if TIMING else "ExternalOutput"
    xT = nc.dram_tensor("xT", [P, KC, T], F16, kind=kin).ap()
    wq = nc.dram_tensor("wq", [P, KC, DG], F16, kind=kin).ap()
    wk = nc.dram_tensor("wk", [P, KC, DG], F16, kind=kin).ap()
    wv = nc.dram_tensor("wv", [P, KC, DG], F16, kind=kin).ap()
    wo = nc.dram_tensor("wo", [P, DT4, D], F16, kind=kin).ap()
    bqs = nc.dram_tensor("bqs", [P, DT4], F32, kind=kin).ap()
    bks = nc.dram_tensor("bks", [P, DT4], F32, kind=kin).ap()
    msk = nc.dram_tensor("msk", [3, P, P], F16, kind=kin).ap()
    out = nc.dram_tensor("out", [T, D], F16, kind=kout).ap()
    tick = (
        nc.dram_tensor("tick", [1, 16], F32, kind="ExternalOutput").ap()
        if TIMING
        else None
    )

    views = dict(
        x0_v=xT[:, :, 0:L],
        x1_v=xT[:, :, L:T],
        wq_v=wq,
        wk_v=wk,
        wv_v=wv,
        wo_v=wo,
        bqs=bqs,
        bks=bks,
        msk=msk,
        out=out,
    )

    with tile.TileContext(nc) as tc:
        with tc.tile_pool(name="persist", bufs=1) as pers:
            nbuf = min(REPEAT, 2)  # x ping-pong kills the WAR stall at
            st = dict(             # REPEAT-iteration boundaries
                x0_bufs=[
                    pers.tile([P, KC, L], F16, name=f"x0_sb{r}") for r in range(nbuf)
                ],
                x1_bufs=[
                    pers.tile([P, KC, L], F16, name=f"x1_sb{r}") for r in range(nbuf)
                ],
                wq_sb=pers.tile([P, KC, DG], F16, name="wq_sb"),
                wk_sb=pers.tile([P, KC, DG], F16, name="wk_sb"),
                wv_sb=pers.tile([P, KC, DG], F16, name="wv_sb"),
                wo_sb=pers.tile([P, DT4, D], F16, name="wo_sb"),
                qT=[pers.tile([P, T], F16, name=f"qT{c}") for c in range(DT4)],
                kT=[pers.tile([P, T], F16, name=f"kT{c}") for c in range(DT4)],
                # v_xt's last reader (diag ctx of the last block) precedes
                # the first cross-rep tail fill, so it needs no ping-pong;
                # v_x0 is read until the very last ctx run -> 2 bufs
                v_xt_bufs=[pers.tile([P, NT, HG * (HD + 1)], F16, name="v_xt")],
                v_x0_bufs=[
                    pers.tile([P, NT, HG * (HD + 1)], F16, name=f"v_x0{r}")
                    for r in range(nbuf)
                ],
                ctxT=pers.tile([P, DT4, T], F16, name="ctxT"),
                bq_sb=pers.tile([P, DT4], F32, name="bq_sb"),
                bk_sb=pers.tile([P, DT4], F32, name="bk_sb"),
                m_strict=pers.tile([P, P], F16, name="m_strict"),
                m_incl=pers.tile([P, P], F16, name="m_incl"),
                m_diag=pers.tile([P, P], F16, name="m_diag"),
                ones_t=pers.tile([P, HD], F16, name="ones_t"),
            )
            nc.vector.memset(st["ones_t"], 1.0)
            if PHASES != "all":  # benign init for phase-subset timing builds
                for tl in st["qT"] + st["kT"] + [st["ctxT"]]:
                    nc.vector.memset(tl, 0.001)
                for vt in st["v_xt_bufs"] + st["v_x0_bufs"]:
                    nc.vector.memset(vt, 1.0)
            for vt in st["v_xt_bufs"] + st["v_x0_bufs"]:
                ones_v = vt.rearrange("p t (h c) -> p (t h) c", c=HD + 1)[
                    :, :, HD : HD + 1
                ]
                nc.vector.memset(ones_v, 1.0)

            if tick is not None:
                tk = pers.tile([1, 16], F32, name="tick_sb")
                nc.vector.memset(tk, 1.0)
                nc.sync.dma_start(tick, tk)

            def _rep_views(r):
                return dict(
                    x0_sb=st["x0_bufs"][r % nbuf],
                    x1_sb=st["x1_bufs"][r % nbuf],
                    v_xt=st["v_xt_bufs"][0],
                    v_x0=st["v_x0_bufs"][r % nbuf],
                )

            for _rep in range(REPEAT):
                st.update(_rep_views(_rep))
                nxt = None
                if _rep + 1 < REPEAT:
                    nxt = _rep_views(_rep + 1)
                _phases(nc, tc, st, views, nxt)

    nc.compile()
    return nc


def _head_dmas(nc, st, views, rv):
    """Per-rep input DMAs (first-needed first; 2 queues). Weights re-DMA
    every rep; masks/biases load once at init."""
    x0_sb, x1_sb = rv["x0_sb"], rv["x1_sb"]
    nc.sync.dma_start(st["bq_sb"], views["bqs"])
    nc.sync.dma_start(st["bk_sb"], views["bks"])
    nc.sync.dma_start(st["m_strict"], views["msk"][0])
    nc.sync.dma_start(st["m_incl"], views["msk"][1])
    nc.sync.dma_start(st["m_diag"], views["msk"][2])
    nc.scalar.dma_start(x0_sb[:, :, 0:256], views["x0_v"][:, :, 0:256])
    nc.sync.dma_start(st["wv_sb"], views["wv_v"])
    nc.scalar.dma_start(x0_sb[:, :, 256:L], views["x0_v"][:, :, 256:L])
    nc.scalar.dma_start(x1_sb, views["x1_v"])
    for c in range(DT4):
        sl = slice(P * c, P * (c + 1))
        nc.sync.dma_start(st["wq_sb"][:, :, sl], views["wq_v"][:, :, sl])
        nc.sync.dma_start(st["wk_sb"][:, :, sl], views["wk_v"][:, :, sl])
    nc.scalar.dma_start(st["wo_sb"], views["wo_v"])


def _v_tiles(nc, st, rv, t2s, scr):
    """V projection tiles; v[t, ch] layout, per-head ones column accumulates
    softmax denominators."""
    for t2 in t2s:
        x_sb = rv["x0_sb"] if t2 < NT else rv["x1_sb"]
        dst = rv["v_xt"] if t2 < NT else rv["v_x0"]
        row = t2 % NT
        toff = P * (t2 % NT)
        ps = scr.tile([P, 1024], F32, tag="ps", name=f"vp{t2}")[:, 0:DG]
        for kc in range(KC):
            nc.tensor.matmul(
                ps,
                x_sb[:, kc, toff : toff + P],
                st["wv_sb"][:, kc, :],
                start=(kc == 0),
                stop=(kc == KC - 1),
            )
        # ACT is idle in the head stretch; keep DVE free for attention work
        nc.scalar.activation(
            dst[:, row].rearrange("p (h c) -> p h c", c=HD + 1)[:, :, :HD],
            ps.rearrange("p (h c) -> p h c", c=HD),
            Act.Copy,
        )


def _head_chunks(nc, st, views, rv, scr):
    """The per-rep prologue as closures: x/weight DMAs, V pass, qk(0).
    For rep r+1 these pop as fillers inside rep r's tail so the next
    iteration's PE work covers the tail's ACT/DMA-bound stalls."""
    chunks = []
    chunks.append(lambda: _head_dmas(nc, st, views, rv))
    if _on("v"):
        for t2s in ((0, 1, 2, 3), (8, 9, 10, 11), (4, 5, 6, 7),
                    (12, 13, 14, 15)):
            chunks.append(lambda t2s=t2s: _v_tiles(nc, st, rv, t2s, scr))
    if _on("qk"):
        for g in range(4):
            chunks.append(lambda g=g: _qk_group(nc, st, 0, g, scr, rv))
    return chunks


def _phases(nc, tc, st, views, nxt):
    ctxT = st["ctxT"]
    rv = {k: st[k] for k in ("x0_sb", "x1_sb", "v_xt", "v_x0")}

    # PSUM: scr ring (2 x 2 banks) + ctx pair (1 x 2 banks) + filler pool
    # (1 x 2 banks) = all 8 banks.
    with (
        tc.tile_pool(name="scr", bufs=2, space="PSUM") as scr,
        tc.tile_pool(name="cps", bufs=1, space="PSUM") as cps,
        tc.tile_pool(name="pj", bufs=1, space="PSUM") as pj,
        tc.tile_pool(name="atp", bufs=14) as atp,
        tc.tile_pool(name="tmp", bufs=2) as tmp,
    ):
        if not st.pop("head_emitted", False):
            for ch in _head_chunks(nc, st, views, rv, scr):
                ch()

        tail_fills = []
        if nxt is not None:
            tail_fills = _head_chunks(nc, st, views, nxt, scr)
            st["head_emitted"] = True

        def tail_fill(k=1):
            for _ in range(k):
                if tail_fills:
                    tail_fills.pop(0)()

        def _oproj_tt(tt, pool, evac):
            ops = pool.tile([P, 1024], F32, tag="ps", name=f"op{tt}")
            for cc in range(DT4):
                stat = ctxT[:, cc, P * tt : P * (tt + 1)]
                for nk in range(2):
                    nc.tensor.matmul(
                        ops[:, 512 * nk : 512 * (nk + 1)],
                        stat,
                        st["wo_sb"][:, cc, 512 * nk : 512 * (nk + 1)],
                        start=(cc == 0),
                        stop=(cc == DT4 - 1),
                    )
            osb = tmp.tile([P, 1024], F16, tag="osb", name=f"osb{tt}")
            if evac == "act":
                nc.scalar.activation(osb, ops, Act.Copy)
            else:
                with nc.allow_low_precision(reason="fp16 out"):
                    nc.vector.tensor_copy(osb, ops)
            # split out-DMA dispatch across SP and the idle Pool queue
            eng = nc.sync if tt % 2 == 0 else nc.gpsimd
            eng.dma_start(views["out"][P * tt : P * (tt + 1), :], osb)

        if _on("attn"):
            blocks = [(c, 0) for c in range(DT4)] + [(c, 1) for c in range(DT4)]
            fills = {}
            for bi, (c, qb) in enumerate(blocks):
                fq = []
                if qb == 0 and c < DT4 - 1 and _on("qk"):
                    # qk(c+1) must fully emit before block c+1's scores
                    fq = [
                        (lambda c=c, g=g: _qk_group(nc, st, c + 1, g, pj, rv))
                        for g in range(4)
                    ]
                elif qb == 1 and c > 0 and _on("oproj"):
                    # tts 0-3 / 8-11 complete once the qb==0 pass (incl. its
                    # last norm_apply) has fully emitted, i.e. from block
                    # (c1, qb1) onward
                    tts = [c - 1, 8 + (c - 1)]
                    if c == DT4 - 1:
                        tts += [c, 8 + c]
                    fq = [
                        (lambda tt=tt: _oproj_tt(tt, pj, "dve"))
                        for tt in tts
                    ]
                fills[bi] = fq
            stages = [
                _attn_stages(nc, st, c, qb, scr, cps, pj, atp, tmp)
                for (c, qb) in blocks
            ]
            # software pipeline: block B+1's ACT-bound score stages emit
            # between block B's PE-dense ctx stages; norm_apply (a PE
            # broadcast gated on the DVE reciprocal) is deferred past the
            # next score stage so PE never waits on the recip latency
            stages[0][0](fills[0], None)     # sc_h0(0)
            stages[0][1](fills[0], None)     # sc_h1(0)
            for bi in range(len(blocks)):
                nxs = stages[bi + 1] if bi + 1 < len(blocks) else None
                stages[bi][2]()              # ctx_h0 (+ recip)
                if nxs:
                    nxs[0](fills[bi + 1], 0)  # sc_h0 part 1 (2 j's)
                stages[bi][3]()              # norm_apply h0
                if nxs:
                    nxs[0](fills[bi + 1], 1)  # sc_h0 rest
                elif _on("oproj"):
                    # xt qb1 spans complete after the last block's h0 norm;
                    # from here on, pops of the next rep's prologue chunks
                    # cover this rep's ACT/DMA-bound tail
                    for i, tt in enumerate((4, 5, 6, 7)):
                        _oproj_tt(tt, scr if i % 2 == 0 else pj, "act")
                        tail_fill(1)
                stages[bi][4]()              # ctx_h1 (+ recip)
                if nxs:
                    nxs[1](fills[bi + 1], 0)
                else:
                    tail_fill(2)
                stages[bi][5]()              # norm_apply h1
                if nxs:
                    nxs[1](fills[bi + 1], 1)
        if _on("oproj"):
            for i, tt in enumerate((12, 13, 14, 15)):
                _oproj_tt(tt, scr if i % 2 == 0 else pj, "act")
                tail_fill(1)
        tail_fill(len(tail_fills))


def _qk_group(nc, st, c, g, pool, rv):
    """One Q/K projection group for head-pair chunk c: g = 2*proj + xhalf
    covers two moving 512-slabs of x[xhalf] against one projection's
    weight chunks, filling one [P, 1024] PSUM slot."""
    proj, pr = divmod(g, 2)
    w_sb, b_sb, dst = [
        (st["wq_sb"], st["bq_sb"], st["qT"][c]),
        (st["wk_sb"], st["bk_sb"], st["kT"][c]),
    ][proj]
    x_sb = rv["x0_sb"] if pr == 0 else rv["x1_sb"]
    slot = pool.tile([P, 1024], F32, tag="ps", name=f"pp{c}_{g}")
    ps = [slot[:, 512 * i : 512 * (i + 1)] for i in range(2)]
    for kc in range(KC):
        stat = w_sb[:, kc, P * c : P * (c + 1)]
        for i in range(2):
            nc.tensor.matmul(
                ps[i],
                stat,
                x_sb[:, kc, 512 * i : 512 * (i + 1)],
                start=(kc == 0),
                stop=(kc == KC - 1),
            )
    for i in range(2):
        dsl = dst[:, L * pr + 512 * i : L * pr + 512 * (i + 1)]
        if c == 0:
            # qk(0) runs before attention: ACT has slack there (q scale
            # folded into Wq; bias-add doubles as the PSUM evacuation)
            nc.scalar.activation(dsl, ps[i], Act.Identity, bias=b_sb[:, c : c + 1])
        else:
            # fillers run inside attention where ACT is saturated with exp
            with nc.allow_low_precision(reason="fp16 qk"):
                nc.vector.tensor_scalar_add(dsl, ps[i], b_sb[:, c : c + 1])


def _attn_stages(nc, st, c, qb, scr, cps, pj, atp, tmp):
    """Sparse BD3LM attention for head pair (2c, 2c+1), one 512-wide q bank,
    as four emission stages: sc_h0, sc_h1 (ACT-bound score runs: K=64
    matmul pairs into [128, 1024] PSUM, even head cols 0-511 / odd
    512-1023, single exp + mask per pair), ctx_h0, ctx_h1 (PE-dense K=128
    ctx runs into a [128, 1024] pair per half; row 64 = softmax denominator
    via the v ones column; followed by normalize). _phases interleaves
    stages of consecutive blocks so ctx runs cover exp latency, and pops
    `fills` closures (next chunk's QK groups / ready O-proj tiles) inside
    score runs to keep PE busy."""
    qTc, kTc, ctxT = st["qT"][c], st["kT"][c], st["ctxT"]
    he, ho = 2 * c, 2 * c + 1
    rows = (slice(0, HD), slice(HD, 2 * HD))
    vcol = (slice((HD + 1) * he, (HD + 1) * (he + 1)),
            slice((HD + 1) * ho, (HD + 1) * (ho + 1)))
    masks = (st["m_strict"], st["m_incl"])
    jmax = 4 if qb == 0 else NT
    ats = {}   # (half, j) -> (at_tile, off)
    atd = []   # diag exp tile (half 0 only)

    def scores(half, fq, part):
        # part: None = all j; 0 = first two j (interleaves before the
        # previous block's norm_apply so its bc matmul never stalls the PE
        # on the reciprocal latency); 1 = the rest
        js = range(jmax)
        if part == 0:
            js = range(2)
        elif part == 1:
            js = range(2, jmax)
        for j in js:
            off = max(0, P * j - 512 * qb)  # start col within this q bank
            sp = scr.tile([P, 1024], F32, tag="ps", name=f"sc{c}{qb}{j}{half}")
            q0 = L * half + 512 * qb
            for e in range(2):
                nc.tensor.matmul(
                    sp[:, 512 * e + off : 512 * (e + 1)],
                    kTc[rows[e], L + P * j : L + P * (j + 1)],
                    qTc[rows[e], q0 + off : q0 + 512],
                    start=True,
                    stop=True,
                )
            if not _x("attn_sc"):
                at = atp.tile([P, 1024], F16, tag="at", name=f"at{c}{qb}{j}{half}")
                sp_v = sp.rearrange("p (e q) -> p e q", e=2)[:, :, off:]
                at_v = at.rearrange("p (e q) -> p e q", e=2)[:, :, off:]
                nc.scalar.activation(at_v, sp_v, Act.Exp)
                if 4 * qb <= j < 4 * (qb + 1):
                    # diag-overlap tile: mask always lands at slice col 0
                    nc.vector.tensor_mul(
                        at_v[:, :, 0:P],
                        at_v[:, :, 0:P],
                        masks[half][:, None, :].to_broadcast((P, 2, P)),
                    )
                ats[(half, j)] = (at, off)
            if j % 2 == 1 and fq:
                fq.pop(0)()
        if part == 0:
            return
        if half == 0:
            if not (_x("attn_sc") or _x("attn_nodiag")):
                atd.append(_diag_scores(nc, st, c, qb, scr, atp, rows, qTc, kTc))
        else:
            while fq:  # all fillers of this block emit before the next block
                fq.pop(0)()

    def ctx_half(half):
        if _x("attn_sc") or _x("attn_scexp"):
            return
        ctx = cps.tile([P, 1024], F32, tag="ctx", name=f"cx{c}{qb}{half}")
        for j in range(jmax):
            at, off = ats[(half, j)]
            for e in range(2):
                nc.tensor.matmul(
                    ctx[0 : HD + 1, 512 * e + off : 512 * (e + 1)],
                    st["v_x0"][:, j, vcol[e]],
                    at[:, 512 * e + off : 512 * (e + 1)],
                    start=(j == 0),
                    stop=(j == jmax - 1),
                )
            if j == 0 and half == 0 and atd:
                # xt-xt diagonal ctx: accumulate between start and stop
                for e in range(2):
                    for i4 in range(4):
                        i = 4 * qb + i4
                        nc.tensor.matmul(
                            ctx[0 : HD + 1, 512 * e + P * i4 : 512 * e + P * (i4 + 1)],
                            st["v_xt"][:, i, vcol[e]],
                            atd[0][:, 512 * e + P * i4 : 512 * e + P * (i4 + 1)],
                            start=False,
                            stop=False,
                        )
        if not _x("attn_nonorm"):
            nrm = tmp.tile(
                [HD + 1, 1024], F16, tag="nrm", name=f"nrm{c}{qb}{half}"
            )
            with nc.allow_low_precision(reason="fp16 recip"):
                nc.vector.reciprocal(nrm[HD : HD + 1, :], ctx[HD : HD + 1, :])
            pend[half] = (ctx, nrm)

    def norm_apply(half):
        if pend[half] is None:
            return
        ctx, nrm = pend[half]
        _norm_apply(nc, st, ctxT, c, half, qb, ctx, nrm, scr, tmp)

    pend = [None, None]
    return (
        lambda fq, part: scores(0, fq, part),
        lambda fq, part: scores(1, fq, part),
        lambda: ctx_half(0),
        lambda: norm_apply(0),
        lambda: ctx_half(1),
        lambda: norm_apply(1),
    )


def _diag_scores(nc, st, c, qb, scr, atp, rows, qTc, kTc):
    """xt-xt block-diagonal score tiles i = 4*qb .. 4*qb+3 for both heads of
    the pair: one [128, 1024] psum pair, one exp, one mask multiply."""
    t = scr.tile([P, 1024], F32, tag="ps", name=f"scd{c}{qb}")
    for i4 in range(4):  # e inner: adjacent matmuls alternate PE row
        i = 4 * qb + i4  # groups 0-63/64-127 and execute concurrently
        for e in range(2):
            nc.tensor.matmul(
                t[:, 512 * e + P * i4 : 512 * e + P * (i4 + 1)],
                kTc[rows[e], P * i : P * (i + 1)],
                qTc[rows[e], P * i : P * (i + 1)],
                start=True,
                stop=True,
            )
    atd = atp.tile([P, 1024], F16, tag="at", name=f"atd{c}{qb}")
    nc.scalar.activation(atd, t, Act.Exp)
    nc.vector.tensor_mul(
        atd.rearrange("p (e i q) -> p e i q", e=2, q=P),
        atd.rearrange("p (e i q) -> p e i q", e=2, q=P),
        st["m_diag"][:, None, None, :].to_broadcast((P, 2, 4, P)),
    )
    return atd


def _norm_apply(nc, st, ctxT, c, half, qb, ctx_pair, nrm, scr, tmp):
    """ctxT rows = ctx[:64] * (1/denom) for one 512-wide q bank, both heads.
    denom reciprocal already in nrm row HD (computed right after ctx stop so
    it is ready by the time the bc matmul issues). Odd head lands on
    partitions 64-127 of ctxT via a staged HWDGE DMA (engines cannot move
    data across partitions)."""
    span = slice(L * half + 512 * qb, L * half + 512 * (qb + 1))
    recip = nrm[HD : HD + 1, :]
    # broadcast 1/denom across 64 partitions on PE (one pair, two banks)
    bcp = scr.tile([P, 1024], F32, tag="ps", name=f"bc{c}{half}{qb}")
    for e in range(2):
        nc.tensor.matmul(
            bcp[0:HD, 512 * e : 512 * (e + 1)],
            st["ones_t"][HD : HD + 1, :],
            recip[:, 512 * e : 512 * (e + 1)],
            start=True,
            stop=True,
        )
    # DVE ops may read at most one PSUM operand: stage bc in SBUF
    rb = nrm[0:HD, :]
    cs = tmp.tile([HD, 512], F16, tag="cs", name=f"cs{c}{half}{qb}")
    with nc.allow_low_precision(reason="fp16 ctx"):
        nc.vector.tensor_copy(rb, bcp[0:HD, :])
        nc.vector.tensor_mul(
            ctxT[:HD, c, span], ctx_pair[0:HD, 0:512], rb[:, 0:512]
        )
        nc.vector.tensor_mul(cs, ctx_pair[0:HD, 512:1024], rb[:, 512:1024])
    # odd head -> ctxT partitions 64-127 (HWDGE on the idle SP queue)
    nc.sync.dma_start(ctxT[HD : 2 * HD, c, span], cs)


def _masks():
    q = np.arange(P)[None, :] // BS
    k = np.arange(P)[:, None] // BS
    m = np.zeros((3, P, P), np.float16)
    m[0] = (q > k).astype(np.float16)    # strict (xt q vs x0 k, same tile)
    m[1] = (q >= k).astype(np.float16)   # incl (x0 q vs x0 k, same tile)
    m[2] = (q == k).astype(np.float16)   # diag (xt q vs xt k, same tile)
    return m


def _swz_k(w):
    """[D, M] -> [P, KC, M] with D = kc*P + p, contiguous per partition."""
    return np.ascontiguousarray(
        w.reshape(KC, P, -1).transpose(1, 0, 2)
    )


def _swz_c(w):
    """[DG, N] -> [P, DT4, N] with DG = cc*P + p."""
    return np.ascontiguousarray(
        w.reshape(DT4, P, -1).transpose(1, 0, 2)
    )


def _in_maps(x, Wq, bq, Wk, bk, Wv, Wo):
    masks = _masks()
    scale = HD ** -0.5
    in_maps = []
    for core in range(8):
        b, g = core // 2, core % 2
        cols = slice(DG * g, DG * (g + 1))
        xb = np.ascontiguousarray(x[b].T).astype(np.float16)  # [D, T]
        in_maps.append(
            {
                "xT": _swz_k(xb),
                "wq": _swz_k(
                    (np.ascontiguousarray(Wq[:, cols]) * np.float32(scale)).astype(
                        np.float16
                    )
                ),
                "wk": _swz_k(np.ascontiguousarray(Wk[:, cols]).astype(np.float16)),
                "wv": _swz_k(np.ascontiguousarray(Wv[:, cols]).astype(np.float16)),
                "wo": _swz_c(np.ascontiguousarray(Wo[cols, :]).astype(np.float16)),
                "bqs": np.ascontiguousarray(
                    (bq[cols].astype(np.float32) * np.float32(scale)).reshape(DT4, P).T
                ),
                "bks": np.ascontiguousarray(
                    bk[cols].astype(np.float32).reshape(DT4, P).T
                ),
                "msk": masks,
            }
        )
    return in_maps


def kernel(x, Wq, bq, Wk, bk, Wv, bv, Wo, bo, block_size=4, **_):
    x = np.asarray(x, np.float32)
    Wq, bq = np.asarray(Wq, np.float32), np.asarray(bq, np.float32)
    Wk, bk = np.asarray(Wk, np.float32), np.asarray(bk, np.float32)
    Wv, bv = np.asarray(Wv, np.float32), np.asarray(bv, np.float32)
    Wo, bo = np.asarray(Wo, np.float32), np.asarray(bo, np.float32)

    if "nc" not in _CACHE:
        _CACHE["nc"] = _build()
    nc = _CACHE["nc"]

    in_maps = _in_maps(x, Wq, bq, Wk, bk, Wv, Wo)
    _CACHE["last_in_maps"] = in_maps
    last_err = None
    for _attempt in range(6):
        try:
            res = run_bass_kernel_spmd(nc, in_maps, core_ids=list(range(8)), trace=False)
            break
        except Exception as e:  # transient NRT device flakes
            last_err = e
            msg = str(e)
            if "UNRECOVERABLE" not in msg and "UNAVAILABLE" not in msg:
                raise
            import time as _time

            import jax as _jax

            _time.sleep(5 * (_attempt + 1))
            try:
                _jax.clear_backends()
            except Exception:
                pass
    else:
        raise last_err

    corr = (bv @ Wo + bo).astype(np.float32)  # softmax rows sum to 1
    out = np.empty((B, T, D), np.float32)
    for b in range(B):
        out[b] = (
            res.results[2 * b]["out"].astype(np.float32)
            + res.results[2 * b + 1]["out"].astype(np.float32)
            + corr
        )
    return out


if __name__ == "__main__":
    rng = np.random.default_rng(0)
    inputs = {
        "x": rng.standard_normal((B, T, D)).astype(np.float32),
        "Wq": (rng.standard_normal((D, D)) / 32).astype(np.float32),
        "bq": np.zeros(D, np.float32),
        "Wk": (rng.standard_normal((D, D)) / 32).astype(np.float32),
        "bk": np.zeros(D, np.float32),
        "Wv": (rng.standard_normal((D, D)) / 32).astype(np.float32),
        "bv": np.zeros(D, np.float32),
        "Wo": (rng.standard_normal((D, D)) / 32).astype(np.float32),
        "bo": np.zeros(D, np.float32),
    }
    o = kernel(**inputs)
    print("ran", o.shape, o.dtype, float(np.abs(o).max()))
